# revision 1
# baseline (speedup 1.0000x reference)
"""ResNet BasicBlock (conv3x3-bn-relu-conv3x3-bn-add-relu) on 8 TRN2 cores.

Data-parallel: batch N=64 split into 8 images per core; conv/BN params
replicated. Each 3x3 conv is computed as 9 shifted [128ci x 128co] bf16
matmuls accumulated in PSUM over a zero-padded [C, 58*58] SBUF image layout
(channels on partitions, padded spatial flattened on the free dim). The host
pre-pads x so each image is one contiguous DMA. Matmul rhs uses strided
valid-column access patterns so no PE cycles are spent on pad columns.

Optimizations vs the 215.6us baseline:
- PE warmup matmuls on a memset zeros tile (no DMA dependency) keep the PE
  busy from the end of the engine preamble until the first input data lands,
  so the HAM clock-gate is at 8/8 (2.4 GHz) for the real matmuls. Sized to
  slightly overshoot the data arrival: an idle gap would restart the HAM
  busy-window and cost 2-4us of 1.2 GHz matmuls.
- Startup DMAs split across both HWDGE rings (sync + scalar engines):
  trigger instructions cost ~600ns each serialized on the issuing engine
  and each ring sustains ~190 GB/s, so w1 taps 3-8 + half of x0's first
  cols ride sync while the other halves ride scalar. Pair 0's tap order
  starts at tap 3 so the first w1 piece suffices.
- Tap-major matmul ordering (both chunks of a pair back-to-back per tap)
  plus a post-schedule pass that drops the second, identical LDWEIGHTS.
- Edge trim (~4.4us PE): output positions whose tap reads only zero padding
  are dropped — one row for dh=+-1 at the image's top/bottom chunk, one
  column per row for dw=+-1 everywhere (strided [rows@56, 55] PSUM dst APs;
  note CoreSim cannot simulate these, hardware handles them fine). Tap
  (0,0) runs first with start=True so PSUM coverage stays complete.
- conv2 epilogue: STT (bn-scale + residual) writes bf16 into a per-image
  [C, 3136] tile, ReLU ACT runs in-place, output DMA is bf16 (half the
  drain), host upcasts to fp32.
- o1_pad pad zeroing via GpSimd memsets so it never queues ahead of the
  PSUM-releasing ACTs on ScalarE.
- params merged into one [C,4] tensor; w2 DMA deferred past startup.

Measured 207.2-207.7us (unthrottled). The device intermittently downclocks
to ~2.0 GHz (P0/thermal, shared device) which adds ~20% to any run,
including the original baseline's.
"""

import numpy as np
from contextlib import ExitStack

import concourse.bass as bass
import concourse.bacc as bacc
import concourse.mybir as mybir
from concourse.tile import TileContext
from concourse.bass_utils import run_bass_kernel_spmd

F32 = mybir.dt.float32
BF16 = mybir.dt.bfloat16
RELU = mybir.ActivationFunctionType.Relu

N_CORES = 8
N_IMG = 8          # images per core
C = 128            # channels (== partitions)
H = W = 56
HP = WP = 58       # padded spatial
S = HP * WP        # 3364 padded flat size
ALLOC = S + 4      # margins so strided valid-col views stay in bounds
HW = H * W         # 3136
# 7 row-chunks of 8 rows; supertiles pair chunks (0,1),(2,3),(4,5),(6)
SUPERS = [(0, 2), (2, 2), (4, 2), (6, 1)]  # (first chunk, n chunks)
NMM = 8 * W        # 448 cols per chunk matmul


def _valid3(t, start, rows):
    """3D [C, rows, 56] valid-column view of padded tile t at alloc offset
    `start` (the alloc index of the first element of the window)."""
    return t[:, start : start + 58 * rows].rearrange("p (r w) -> p r w", w=58)[
        :, :, 0:56
    ]


def _ps4(ps, nchunk, rows=8):
    """[C, nchunk, rows, 56] view of a [C,1024] psum supertile: chunk slots at
    512-col alignment, 448 valid cols each."""
    return (
        ps[:, 0 : 512 * nchunk]
        .rearrange("p (s x) -> p s x", x=512)[:, :, 0 : rows * 56]
        .rearrange("p s (r w) -> p s r w", w=56)
    )


def _pad4(t, vbase, nchunk, rows=8):
    """[C, nchunk, rows, 56] valid view of padded image tile at vbase."""
    return (
        t[:, vbase : vbase + 58 * 8 * nchunk]
        .rearrange("p (s r w) -> p s r w", s=nchunk, w=58)[:, :, 0:rows, 0:56]
    )


def _flat4(t, col0, nchunk, rows=8):
    """[C, nchunk, rows, 56] view of a flat [C,3136] tile from col0."""
    return (
        t[:, col0 : col0 + 448 * nchunk]
        .rearrange("p (s r w) -> p s r w", s=nchunk, w=56)[:, :, 0:rows, :]
    )


def _zero_pads_gpsimd(nc, t):
    """Zero every padded position of a [128, ALLOC] image tile on GpSimd.

    GpSimd so the zeroing never queues ahead of conv ACTs on ScalarE (the
    ACTs release PSUM supertiles; any delay there stalls the PE).
    """
    nc.gpsimd.memset(t[:, 0:60], 0.0)
    pairs = t[:, 58 : 58 + 57 * 58].rearrange("p (r w) -> p r w", w=58)[:, :, 0:2]
    nc.gpsimd.memset(pairs, 0.0)
    nc.gpsimd.memset(t[:, 3307:ALLOC], 0.0)


def _conv_pair(nc, ps_tiles, w_sb, src, sc, nchunk):
    """Accumulate a pair of row-chunks: 9 taps x nchunk matmuls, tap-major so
    the two chunks' matmuls for one tap are back-to-back (the second skips
    its LDWEIGHTS via _dedupe_ldweights). Each chunk has its own single-bank
    PSUM tile in ps_tiles, so the consumer releases banks per chunk.

    Edge trim: matmul output positions whose tap only reads zero padding are
    dropped — one row for dh=+-1 on the image's top/bottom chunk, one column
    per row for dw=+-1 everywhere. Tap 4 (dh=dw=0, full 448-col coverage)
    goes first and carries start=True, so every PSUM position is cleared /
    freshly written before any trimmed tap accumulates.
    """
    chunks = [sc + i for i in range(nchunk)]
    tap_order = [4, 3, 5, 6, 7, 8, 0, 1, 2] if sc == 0 else [4, 0, 1, 2, 3, 5, 6, 7, 8]
    n_done = {c: 0 for c in chunks}
    for t in tap_order:
        dh, dw = t // 3 - 1, t % 3 - 1
        for i, c in enumerate(chunks):
            ps = ps_tiles[i]
            vbase = (1 + 8 * c) * WP + 2
            n_done[c] += 1
            # output row range [r0, r0+nr) of the 8-row chunk
            r0, nr = 0, 8
            if c == 0 and dh == -1:
                r0, nr = 1, 7
            elif c == 6 and dh == 1:
                r0, nr = 0, 7
            # output col range [c0, c0+ncol) of each 56-col row
            c0, ncol = (1, 55) if dw == -1 else (0, 55 if dw == 1 else 56)
            off = r0 * 56 + c0
            out = ps[:, off : off + 56 * nr].rearrange("p (r w) -> p r w", w=56)[
                :, :, 0:ncol
            ]
            start_idx = vbase + (r0 + dh) * WP + c0 + dw
            rhs = src[:, start_idx : start_idx + 58 * nr].rearrange(
                "p (r w) -> p r w", w=58
            )[:, :, 0:ncol]
            nc.tensor.matmul(
                out,
                w_sb[:, t * C : (t + 1) * C],
                rhs,
                start=(t == 4),
                stop=n_done[c] == 9,
            )


def _dedupe_ldweights(nc):
    """Drop an InstLdweights whose stationary operand is already loaded (the
    previous PE Ldweights had an identical access pattern and only Matmults
    ran since). Waits/updates of a dropped Ldweights transfer to the next PE
    instruction so semaphore semantics are preserved.

    Runs after tile scheduling, before Bacc.compile().
    """

    def merge_syncs(pending, inst):
        if pending is None:
            return
        si = inst.sync_info
        if si is None:
            inst.sync_info = pending
        else:
            si.on_wait = list(pending.on_wait) + list(si.on_wait)
            si.on_update = list(pending.on_update) + list(si.on_update)

    removed = 0
    for f in nc.m.functions:
        for bb in f.blocks:
            last_key = None
            pending = None  # syncs of a dropped Ldweights
            out = []
            for inst in bb.instructions:
                if "PE" not in str(getattr(inst, "engine", "")):
                    out.append(inst)
                    continue
                nm = type(inst).__name__
                if nm == "InstLdweights":
                    k = repr(inst.ins[0])
                    if k == last_key:
                        si = inst.sync_info
                        if si is not None and (si.on_wait or si.on_update):
                            if pending is None:
                                pending = si
                            else:
                                pending.on_wait = list(pending.on_wait) + list(
                                    si.on_wait
                                )
                                pending.on_update = list(pending.on_update) + list(
                                    si.on_update
                                )
                        removed += 1
                        continue
                    last_key = k
                elif nm != "InstMatmult":
                    last_key = None
                merge_syncs(pending, inst)
                pending = None
                out.append(inst)
            assert pending is None
            bb.instructions[:] = out
    return removed


SEM_POOL_STOP = 0  # shrink kernel sem pool: NEFF teardown resets one
                     # instruction per referenced semaphore (~115ns each)


def build_module(n_img=N_IMG):
    orig_range = bass.get_kernel_semaphore_range
    if SEM_POOL_STOP:
        start = orig_range().start
        bass.get_kernel_semaphore_range = lambda: range(start, SEM_POOL_STOP)
    try:
        nc = bacc.Bacc()
    finally:
        bass.get_kernel_semaphore_range = orig_range

    x_d = nc.dram_tensor("x", [n_img, C, ALLOC], BF16, kind="ExternalInput")
    w1_d = nc.dram_tensor("w1t", [C, 9 * C], BF16, kind="ExternalInput")
    w2_d = nc.dram_tensor("w2t", [C, 9 * C], BF16, kind="ExternalInput")
    prm_d = nc.dram_tensor("prm", [C, 4], F32, kind="ExternalInput")
    out_d = nc.dram_tensor("out", [n_img, C, HW], BF16, kind="ExternalOutput")

    with TileContext(nc) as tc, ExitStack() as ctx:
        wpool = ctx.enter_context(tc.tile_pool(name="wpool", bufs=1))
        xpool = ctx.enter_context(tc.tile_pool(name="xpool", bufs=4))
        o1pool = ctx.enter_context(tc.tile_pool(name="o1pool", bufs=3))
        t1pool = ctx.enter_context(tc.tile_pool(name="t1pool", bufs=3))
        ps1pool = ctx.enter_context(tc.tile_pool(name="ps1", bufs=4, space="PSUM"))
        ps2pool = ctx.enter_context(tc.tile_pool(name="ps2", bufs=4, space="PSUM"))

        w1_sb = wpool.tile([C, 9 * C], BF16, name="w1_sb")
        w2_sb = wpool.tile([C, 9 * C], BF16, name="w2_sb")
        prm_sb = wpool.tile([C, 4], F32, name="prm_sb")
        s1_sb, h1_sb = prm_sb[:, 0:1], prm_sb[:, 1:2]
        s2_sb, h2_sb = prm_sb[:, 2:3], prm_sb[:, 3:4]
        wz = wpool.tile([C, 512], BF16, name="wz")
        # gpsimd clears its engine preamble earliest; using it for the memset
        # lets the PE warmup start ~1us sooner
        nc.gpsimd.memset(wz[:, :], 0.0)

        # Warm up the PE HAM clock gate immediately (no DMA dependency):
        # 512-col matmuls on zeros keep PE busy from t~7us until the first
        # input data lands (~9.5us), so real matmuls run at the warm clock.
        psw = ps1pool.tile([C, 448], F32, name="ps1_t", tag="ps1_t")
        for i in range(8):
            nc.tensor.matmul(
                psw[:, :], wz[:, 0:128], wz[:, 0:448],
                start=(i == 0), stop=(i == 7),
            )

        # Startup DMAs split across both HWDGE rings (sync + scalar): trigger
        # instructions cost ~600ns each on the issuing engine, transfers are
        # FIFO per ring, and each ring sustains only ~190 GB/s. The first
        # matmuls gate on x0 cols 0..1056 (sync ring, first in line) || w1
        # taps 3-8 (scalar ring; pair 0's tap order starts at tap 3).
        nc.scalar.dma_start(w1_sb[:, 384:1152], w1_d[:, 384:1152])
        nc.scalar.dma_start(w1_sb[:, 0:384], w1_d[:, 0:384])

        def issue_x(img, cuts=None, engines=None):
            # split the image DMA so the first chunks' matmuls can start
            # before the whole image has landed
            x_pad = xpool.tile([C, ALLOC], BF16, name="x_pad")
            cuts = cuts or [0, ALLOC // 2, ALLOC]
            for i, (a, b) in enumerate(zip(cuts, cuts[1:])):
                eng = engines[i] if engines else nc.sync
                eng.dma_start(x_pad[:, a:b], x_d[img, :, a:b])
            return x_pad

        x_tiles = [None] * n_img
        # x0 cols 0..1056 lead the sync ring (pair 0's input); the rest of
        # x0 follows behind the weights on the scalar ring
        x_tiles[0] = issue_x(
            0,
            cuts=[0, 1056, 2112, ALLOC],
            engines=[nc.sync, nc.scalar, nc.scalar],
        )
        nc.sync.dma_start(prm_sb[:, :], prm_d[:, :])

        for img in range(n_img):
            # prefetch next image's input one iteration ahead so it is never
            # queued behind this image's output DMAs
            if img + 1 < n_img:
                x_tiles[img + 1] = issue_x(img + 1)
            if img == 0:
                # off the startup critical path: w2 is first needed ~25us in
                nc.sync.dma_start(w2_sb[:, :], w2_d[:, :])
            x_pad = x_tiles[img]

            # o1_pad is written only by ScalarE: pad zeroing first, then the
            # per-supertile bn+relu writes of the valid columns.
            o1_pad = o1pool.tile([C, ALLOC], BF16, name="o1_pad")
            _zero_pads_gpsimd(nc, o1_pad)

            # conv1 + bn1 + relu -> o1_pad
            for sc, nchunk in SUPERS:
                pss = [
                    ps1pool.tile([C, NMM + 8], F32, name="ps1_t", tag="ps1_t")
                    for _ in range(nchunk)
                ]
                _conv_pair(nc, pss, w1_sb, x_pad, sc, nchunk)
                for i in range(nchunk):
                    c = sc + i
                    vbase = (1 + 8 * c) * WP + 2
                    nc.scalar.activation(
                        _valid3(o1_pad, vbase, 8),
                        pss[i][:, 0:NMM].rearrange("p (r w) -> p r w", w=56),
                        RELU, bias=h1_sb, scale=s1_sb,
                    )

            # conv2 + bn2 + residual + relu -> t1 (bf16), DMA out per super
            t1 = t1pool.tile([C, HW], BF16, name="t1")
            for sc, nchunk in SUPERS:
                pss = [
                    ps2pool.tile([C, NMM + 8], F32, name="ps2_t", tag="ps2_t")
                    for _ in range(nchunk)
                ]
                _conv_pair(nc, pss, w2_sb, o1_pad, sc, nchunk)
                if img == n_img - 1 and sc == 6:
                    # The very last chunk: pipeline its postprocessing in two
                    # 4-row halves so the STT/ACT/trigger/transfer chain that
                    # gates the end of the kernel is ~0.5us shorter.
                    vbase = (1 + 8 * 6) * WP + 2
                    for h in range(2):
                        cc = 448 * 6 + 224 * h
                        nc.vector.scalar_tensor_tensor(
                            t1[:, cc : cc + 224].rearrange(
                                "p (r w) -> p r w", w=56
                            ),
                            pss[0][:, 224 * h : 224 * (h + 1)].rearrange(
                                "p (r w) -> p r w", w=56
                            ),
                            s2_sb,
                            _valid3(x_pad, vbase + 4 * h * WP, 4),
                            op0=mybir.AluOpType.mult, op1=mybir.AluOpType.add,
                        )
                        seg = t1[:, cc : cc + 224]
                        nc.scalar.activation(
                            seg, seg, RELU, bias=h2_sb, scale=1.0
                        )
                        # the two final triggers ride different rings so they
                        # don't serialize behind each other; the very last one
                        # issues from ScalarE itself right after its own ACT
                        # (no later ACT exists to delay, and it skips the
                        # cross-engine semaphore hop)
                        eng = nc.gpsimd if h == 0 else nc.scalar
                        eng.dma_start(out_d[img, :, cc : cc + 224], seg)
                    continue
                for i in range(nchunk):
                    c = sc + i
                    vbase = (1 + 8 * c) * WP + 2
                    # VectorE: t1 = conv2*scale2 + x (residual), PSUM -> bf16
                    nc.vector.scalar_tensor_tensor(
                        t1[:, 448 * c : 448 * (c + 1)].rearrange(
                            "p (r w) -> p r w", w=56
                        ),
                        pss[i][:, 0:NMM].rearrange("p (r w) -> p r w", w=56),
                        s2_sb,
                        _valid3(x_pad, vbase, 8),
                        op0=mybir.AluOpType.mult, op1=mybir.AluOpType.add,
                    )
                col0 = 448 * sc
                # ScalarE in-place: t1 = relu(t1 + shift2)
                seg = t1[:, col0 : col0 + 448 * nchunk]
                nc.scalar.activation(seg, seg, RELU, bias=h2_sb, scale=1.0)
                nc.gpsimd.dma_start(
                    out_d[img, :, col0 : col0 + 448 * nchunk], seg
                )

    n_removed = _dedupe_ldweights(nc)
    assert n_removed >= 54 * n_img, f"ldweights dedupe removed only {n_removed}"
    nc.compile()
    return nc


EPS = 1e-5


def _prep_params(w1, g1, b1, m1, v1, w2, g2, b2, m2, v2):
    s1 = (g1 / np.sqrt(v1 + EPS)).astype(np.float32)
    h1 = (b1 - m1 * s1).astype(np.float32)
    s2 = (g2 / np.sqrt(v2 + EPS)).astype(np.float32)
    h2 = (b2 - m2 * s2).astype(np.float32)
    prm = np.stack([s1, h1, s2, h2], axis=1).astype(np.float32)  # [C,4]
    # w[o, i, kh, kw] -> [i, (kh*3+kw)*128 + o]
    import ml_dtypes

    w1t = np.ascontiguousarray(w1.transpose(1, 2, 3, 0).reshape(C, 9 * C)).astype(
        ml_dtypes.bfloat16
    )
    w2t = np.ascontiguousarray(w2.transpose(1, 2, 3, 0).reshape(C, 9 * C)).astype(
        ml_dtypes.bfloat16
    )
    return w1t, w2t, prm


def pad_images(x):
    """[n, C, 56, 56] -> bf16 [n, C, ALLOC] zero-padded 58x58 + margins."""
    import ml_dtypes

    n = x.shape[0]
    buf = np.zeros((n, C, ALLOC), dtype=ml_dtypes.bfloat16)
    v = buf[:, :, 60 : 60 + 58 * 56].reshape(n, C, 56, 58)
    v[:, :, :, :56] = x.astype(ml_dtypes.bfloat16)
    return buf


def kernel(x, w1, g1, b1, m1, v1, w2, g2, b2, m2, v2):
    x = np.asarray(x, dtype=np.float32)
    n = x.shape[0]
    assert n == N_CORES * N_IMG, x.shape
    w1t, w2t, prm = _prep_params(
        np.asarray(w1), np.asarray(g1), np.asarray(b1), np.asarray(m1), np.asarray(v1),
        np.asarray(w2), np.asarray(g2), np.asarray(b2), np.asarray(m2), np.asarray(v2),
    )
    xp = pad_images(x.reshape(n, C, H, W))
    nc = build_module()
    in_maps = []
    for cid in range(N_CORES):
        xs = np.ascontiguousarray(xp[cid * N_IMG : (cid + 1) * N_IMG])
        in_maps.append({"x": xs, "w1t": w1t, "w2t": w2t, "prm": prm})
    res = run_bass_kernel_spmd(nc, in_maps, core_ids=list(range(N_CORES)))
    out = np.concatenate(
        [np.asarray(r["out"], dtype=np.float32) for r in res.results], axis=0
    )
    return out.reshape(n, C, H, W)

